# revision 1
# baseline (speedup 1.0000x reference)
"""Causal self-attention kernel for 8 Trainium2 NeuronCores (Bass/Tile).

Problem: y = CausalSelfAttention(x) with B=4, T=2048, C=1024, 16 heads.
Sharding: 8 cores = 4 batches x 2 head-groups (8 heads each); each core
computes its batch's attention for its head group plus the partial output
projection; the host sums the two partials per batch.
"""


import numpy as np
import concourse.bass as bass
import concourse.tile as tile
from concourse import mybir, bacc

F32 = mybir.dt.float32
F32R = mybir.dt.float32r
BF16 = mybir.dt.bfloat16


def build(T=2048, HL=8, C=1024):
    """Build the per-core SPMD program. HL = heads handled by this core."""
    D = 64
    HP = HL // 2               # head pairs
    NCK = C // 128             # contraction chunks for qkv
    NI = T // 512              # 512-wide token blocks
    NTK = T // 128             # 128-wide key blocks

    nc = bacc.Bacc("TRN2", debug=False, num_devices=8)

    xt = nc.dram_tensor("xt", [NCK, 128, T], F32R, kind="ExternalInput")
    wq = nc.dram_tensor("wq", [NCK, 128, HL * D], F32R, kind="ExternalInput")
    wk = nc.dram_tensor("wk", [NCK, 128, HL * D], F32R, kind="ExternalInput")
    wv = nc.dram_tensor("wv", [NCK, 128, HL * D], F32R, kind="ExternalInput")
    wp = nc.dram_tensor("wp", [HP, 128, C], BF16, kind="ExternalInput")
    tri = nc.dram_tensor("tri", [128, 256], BF16, kind="ExternalInput")
    ident = nc.dram_tensor("ident", [128, 128], BF16, kind="ExternalInput")
    one64 = nc.dram_tensor("one64", [1, 64], F32R, kind="ExternalInput")
    out = nc.dram_tensor("out", [T, C], F32, kind="ExternalOutput")

    with tile.TileContext(nc) as tc:
        with (
            tc.tile_pool(name="persist", bufs=1) as pers,
            tc.tile_pool(name="qkv", bufs=1) as qkvp,
        ):
            id_sb = pers.tile([128, 128], BF16, tag="ident")
            nc.sync.dma_start(id_sb[:], ident[:])

            q_sb = qkvp.tile([128, HP, T], F32R, tag="q")
            k_sb = qkvp.tile([128, HP, T], F32R, tag="k")
            v_sb = qkvp.tile([128, HP, NTK, 130], BF16, tag="v")
            nc.vector.memset(v_sb[:], 1.0)

            # ---- phase A: q^T, k^T, v for all head pairs ----
            with (
                tc.tile_pool(name="xtp", bufs=1) as xtp,
                tc.tile_pool(name="wst", bufs=3) as wst,
                tc.tile_pool(name="vtb", bufs=2) as vtp,
                tc.tile_pool(name="ps_a", bufs=6, space="PSUM") as ps_a,
                tc.tile_pool(name="ps_tr", bufs=2, space="PSUM") as ps_tr,
            ):
                # first head-pair's weights before the big x DMA so PE can start
                w_tiles = []
                for hp in range(HP):
                    hs = slice(hp * 128, hp * 128 + 128)
                    wq_h = wst.tile([128, NCK, 128], F32R, tag=f"w{hp}", name=f"wq{hp}")
                    wk_h = wst.tile([128, NCK, 128], F32R, tag=f"w{hp}", name=f"wk{hp}")
                    wv_h = wst.tile([128, NCK, 128], F32R, tag=f"w{hp}", name=f"wv{hp}")
                    nc.sync.dma_start(wq_h[:], wq[:, :, hs].transpose([1, 0, 2]))
                    nc.sync.dma_start(wk_h[:], wk[:, :, hs].transpose([1, 0, 2]))
                    nc.sync.dma_start(wv_h[:], wv[:, :, hs].transpose([1, 0, 2]))
                    w_tiles.append((wq_h, wk_h, wv_h))
                    if hp == 0:
                        xt_sb = xtp.tile([128, NCK, T], F32R, tag="xt")
                        for ck in range(NCK):
                            nc.sync.dma_start(xt_sb[:, ck, 0:T // 2], xt[ck, :, 0:T // 2])
                            nc.sync.dma_start(xt_sb[:, ck, T // 2:T], xt[ck, :, T // 2:T])
                for hp in range(HP):
                    wq_h, wk_h, wv_h = w_tiles[hp]
                    for i in range(NI):
                        ts = slice(512 * i, 512 * i + 512)
                        pq = ps_a.tile([128, 512], F32, tag="mm")
                        for ck in range(NCK):
                            nc.tensor.matmul(pq[:], wq_h[:, ck, :], xt_sb[:, ck, ts],
                                             start=(ck == 0), stop=(ck == NCK - 1))
                        nc.vector.tensor_copy(q_sb[:, hp, ts], pq[:])
                        pk = ps_a.tile([128, 512], F32, tag="mm")
                        for ck in range(NCK):
                            nc.tensor.matmul(pk[:], wk_h[:, ck, :], xt_sb[:, ck, ts],
                                             start=(ck == 0), stop=(ck == NCK - 1))
                        nc.vector.tensor_copy(k_sb[:, hp, ts], pk[:])
                        pv = ps_a.tile([128, 512], F32, tag="mm")
                        for ck in range(NCK):
                            nc.tensor.matmul(pv[:], wv_h[:, ck, :], xt_sb[:, ck, ts],
                                             start=(ck == 0), stop=(ck == NCK - 1))
                        vt_bf = vtp.tile([128, 512], BF16, tag="vt")
                        nc.vector.tensor_copy(vt_bf[:], pv[:])
                        for f in range(4):
                            pt = ps_tr.tile([128, 128], BF16, tag="tr")
                            nc.tensor.transpose(pt[:], vt_bf[:, 128 * f:128 * f + 128],
                                                id_sb[:])
                            nc.vector.tensor_copy(v_sb[:, hp, 4 * i + f, 0:64],
                                                  pt[:, 0:64])
                            nc.vector.tensor_copy(v_sb[:, hp, 4 * i + f, 65:129],
                                                  pt[:, 64:128])

            # ---- phases B + C ----
            with (
                tc.tile_pool(name="yp", bufs=1) as yp,
                tc.tile_pool(name="cst", bufs=1) as cst,
                tc.tile_pool(name="att", bufs=12) as attp,
                tc.tile_pool(name="rec", bufs=2) as recp,
                tc.tile_pool(name="den", bufs=1) as denp,
                tc.tile_pool(name="outp", bufs=3) as outp,
                tc.tile_pool(name="ps_s", bufs=2, space="PSUM") as ps_s,
                tc.tile_pool(name="ps_yd", bufs=1, space="PSUM") as ps_yd,
                tc.tile_pool(name="ps_pr", bufs=2, space="PSUM") as ps_pr,
            ):
                y_sb = yp.tile([128, HP, T], BF16, tag="y")
                wp_sb = cst.tile([128, HP, C], BF16, tag="wp")
                for hp in range(HP):
                    nc.sync.dma_start(wp_sb[:, hp, :], wp[hp])
                tri_sb = cst.tile([128, 256], BF16, tag="tri")
                nc.sync.dma_start(tri_sb[:], tri[:])

                for j in range(NI):
                    tqs = slice(512 * j, 512 * j + 512)
                    ntk = 4 * j + 4
                    for hp in range(HP):
                        pyd = ps_yd.tile([128, 1024], F32, tag="yd")
                        for tkb in range(ntk):
                            ks = slice(128 * tkb, 128 * tkb + 128)
                            pss = ps_s.tile([128, 1024], F32, tag="s")
                            nc.tensor.matmul(pss[:, 0:512], k_sb[0:64, hp, ks],
                                             q_sb[0:64, hp, tqs],
                                             start=True, stop=True, tile_position=(0, 0))
                            nc.tensor.matmul(pss[:, 512:1024], k_sb[64:128, hp, ks],
                                             q_sb[64:128, hp, tqs],
                                             start=True, stop=True, tile_position=(64, 0))
                            att = attp.tile([128, 2, 512], BF16, tag="att")
                            r = tkb - 4 * j
                            if r < 0:
                                nc.scalar.activation(
                                    att[:], pss[:].rearrange("p (h t) -> p h t", h=2),
                                    mybir.ActivationFunctionType.Exp, scale=0.125)
                            else:
                                if r > 0:
                                    nc.vector.memset(att[:, :, 0:128 * r], 0.0)
                                nc.scalar.activation(
                                    att[:, :, 128 * r:512],
                                    pss[:].rearrange("p (h t) -> p h t", h=2)[:, :, 128 * r:512],
                                    mybir.ActivationFunctionType.Exp, scale=0.125)
                                nc.vector.tensor_mul(
                                    att[:, :, 128 * r:128 * r + 128],
                                    att[:, :, 128 * r:128 * r + 128],
                                    tri_sb[:].rearrange("p (h t) -> p h t", h=2))
                            st = (tkb == 0)
                            sp = (tkb == ntk - 1)
                            nc.tensor.matmul(pyd[0:65, 0:512], v_sb[:, hp, tkb, 0:65],
                                             att[:, 0, :], start=st, stop=sp)
                            nc.tensor.matmul(pyd[0:65, 512:1024],
                                             v_sb[:, hp, tkb, 65:130],
                                             att[:, 1, :], start=st, stop=sp)
                        # stage unnormalized y + denominators out of PSUM quickly
                        den = denp.tile([65, 1024], F32, tag="den")
                        nc.vector.tensor_copy(den[64:65, :], pyd[64:65, :])
                        yu = recp.tile([64, 1024], BF16, tag="yu")
                        nc.vector.tensor_copy(yu[:], pyd[0:64, :])
                        den0 = denp.tile([1, 1024], F32, tag="den0")
                        nc.sync.dma_start(den0[:], den[64:65, :])
                        dT = denp.tile([64, 1024], F32, tag="dT")
                        nc.gpsimd.partition_broadcast(dT[:], den0[0:1, :])
                        recT = recp.tile([128, 512], F32, tag="recT")
                        nc.sync.dma_start(recT[0:64, :], dT[0:64, 0:512])
                        nc.sync.dma_start(recT[64:128, :], dT[0:64, 512:1024])
                        nc.vector.reciprocal(recT[:], recT[:])
                        nc.vector.tensor_mul(y_sb[0:64, hp, tqs], yu[:, 0:512],
                                             recT[0:64, :])
                        nc.sync.dma_start(y_sb[64:128, hp, tqs], yu[:, 512:1024])
                        nc.vector.tensor_mul(y_sb[64:128, hp, tqs],
                                             y_sb[64:128, hp, tqs], recT[64:128, :])

                    # ---- phase C: projection for this finished tq block ----
                    for f in range(4):
                        t = 4 * j + f
                        ysl = slice(128 * t, 128 * t + 128)
                        ot = outp.tile([128, C], F32, tag="ot")
                        for ch in range(C // 512):
                            po = ps_pr.tile([128, 512], F32, tag="po")
                            for hp in range(HP):
                                nc.tensor.matmul(po[:], y_sb[:, hp, ysl],
                                                 wp_sb[:, hp, 512 * ch:512 * ch + 512],
                                                 start=(hp == 0), stop=(hp == HP - 1))
                            nc.vector.tensor_copy(ot[:, 512 * ch:512 * ch + 512], po[:])
                        nc.sync.dma_start(out[128 * t:128 * t + 128, :], ot[:])

    nc.compile()
    return nc


def make_inputs(x_b, w_qkv, w_proj, g, HL=8):
    """Host-side prep of one core's input map.

    x_b: [T, C] fp32 (one batch), g: head-group index (0 or 1).
    """
    import ml_dtypes
    T, C = x_b.shape
    D = 64
    NCK = C // 128
    HP = HL // 2
    h0 = g * HL * D
    xt = np.ascontiguousarray(x_b.T.reshape(NCK, 128, T))
    wq = np.ascontiguousarray(w_qkv[:, h0:h0 + HL * D].reshape(NCK, 128, HL * D))
    wk = np.ascontiguousarray(w_qkv[:, C + h0:C + h0 + HL * D].reshape(NCK, 128, HL * D))
    wv = np.ascontiguousarray(w_qkv[:, 2 * C + h0:2 * C + h0 + HL * D].reshape(NCK, 128, HL * D))
    wp = np.ascontiguousarray(
        w_proj[h0:h0 + HL * D, :].reshape(HP, 128, C)).astype(ml_dtypes.bfloat16)
    t1 = np.triu(np.ones((128, 128), dtype=np.float32))
    tri = np.concatenate([t1, t1], axis=1).astype(ml_dtypes.bfloat16)
    ident = np.eye(128, dtype=np.float32).astype(ml_dtypes.bfloat16)
    return {"xt": xt, "wq": wq, "wk": wk, "wv": wv, "wp": wp,
            "tri": tri, "ident": ident, "one64": np.ones((1, 64), dtype=np.float32)}


_NC_CACHE = {}


def kernel(x, w_qkv, w_proj):
    import numpy as np
    from concourse.bass_utils import run_bass_kernel_spmd

    x = np.ascontiguousarray(np.asarray(x, dtype=np.float32))
    w_qkv = np.ascontiguousarray(np.asarray(w_qkv, dtype=np.float32))
    w_proj = np.ascontiguousarray(np.asarray(w_proj, dtype=np.float32))
    B, T, C = x.shape

    key = (T, C)
    if key not in _NC_CACHE:
        _NC_CACHE[key] = build(T=T, HL=8, C=C)
    nc = _NC_CACHE[key]

    in_maps = [make_inputs(x[c // 2], w_qkv, w_proj, c % 2, HL=8) for c in range(8)]
    res = run_bass_kernel_spmd(nc, in_maps, core_ids=list(range(8)), trace=False)

    out = np.zeros((B, T, C), dtype=np.float32)
    for c in range(8):
        out[c // 2] += res.results[c]["out"]
    return out



# revision 6
# speedup vs baseline: 1.2026x; 1.2026x over previous
"""Causal self-attention kernel for 8 Trainium2 NeuronCores (Bass/Tile).

Problem: y = CausalSelfAttention(x) with B=4, T=2048, C=1024, 16 heads.
Sharding: 8 cores = 4 batches x 2 head-groups (8 heads each); each core
computes its batch's attention for its head group plus the partial output
projection; the host sums the two partials per batch.

v2: all-bf16 inputs, Act-engine PSUM drains, fast reciprocal normalize,
proj interleaved into the attention j-loop, PSUM ring shared yd/proj.
"""


import numpy as np
import concourse.bass as bass
import concourse.tile as tile
from concourse import mybir, bacc

F32 = mybir.dt.float32
BF16 = mybir.dt.bfloat16
EXP = mybir.ActivationFunctionType.Exp
COPY = mybir.ActivationFunctionType.Copy


def build(T=2048, HL=8, C=1024):
    """Build the per-core SPMD program. HL = heads handled by this core."""
    D = 64
    HP = HL // 2               # head pairs
    NCK = C // 128             # contraction chunks for qkv
    NI = T // 512              # 512-wide token blocks
    NTK = T // 128             # 128-wide key blocks

    nc = bacc.Bacc("TRN2", debug=False, num_devices=8)

    xt = nc.dram_tensor("xt", [NCK, 128, T], BF16, kind="ExternalInput")
    wqkv = nc.dram_tensor("wqkv", [3, 128, HP, NCK, 128], BF16,
                          kind="ExternalInput")
    wp = nc.dram_tensor("wp", [HP, 128, C], BF16, kind="ExternalInput")
    tri = nc.dram_tensor("tri", [128, 256], BF16, kind="ExternalInput")
    ident = nc.dram_tensor("ident", [128, 128], BF16, kind="ExternalInput")
    out = nc.dram_tensor("out", [T, C], BF16, kind="ExternalOutput")

    with tile.TileContext(nc) as tc:
        with tc.tile_pool(name="persist", bufs=1) as pers:
            id_sb = pers.tile([128, 128], BF16, tag="ident")
            nc.sync.dma_start(id_sb[:], ident[:])
            tri_sb = pers.tile([128, 256], BF16, tag="tri")
            wp_sb = pers.tile([128, HP, C], BF16, tag="wp")
            q_sb = pers.tile([128, HP, T], BF16, tag="q")
            k_sb = pers.tile([128, HP, T], BF16, tag="k")
            # v^T per key block: [keys, hp, tkb, head, 64 dims + ones col]
            v_sb = pers.tile([128, HP, NTK, 2, 65], BF16, tag="v")
            y_sb = pers.tile([128, HP, T], BF16, tag="y")
            nc.vector.memset(v_sb[:, :, :, :, 64:65], 1.0)

            # ---- phase A: q^T, k^T, v for all head pairs ----
            with (
                tc.tile_pool(name="xtp", bufs=1) as xtp,
                tc.tile_pool(name="wst", bufs=3) as wst,
                tc.tile_pool(name="vtb", bufs=2) as vtp,
                tc.tile_pool(name="ps_a", bufs=6, space="PSUM") as ps_a,
                tc.tile_pool(name="ps_tr", bufs=2, space="PSUM") as ps_tr,
            ):
                xt_sb = xtp.tile([128, NCK, T], BF16, tag="xt")
                w_tiles = []
                for hp in range(HP):
                    wq_h = wst.tile([128, NCK, 128], BF16, tag=f"w{hp}",
                                    name=f"wq{hp}")
                    wk_h = wst.tile([128, NCK, 128], BF16, tag=f"w{hp}",
                                    name=f"wk{hp}")
                    wv_h = wst.tile([128, NCK, 128], BF16, tag=f"w{hp}",
                                    name=f"wv{hp}")
                    nc.sync.dma_start(wq_h[:], wqkv[0, :, hp])
                    nc.sync.dma_start(wk_h[:], wqkv[1, :, hp])
                    nc.sync.dma_start(wv_h[:], wqkv[2, :, hp])
                    w_tiles.append((wq_h, wk_h, wv_h))
                    if hp == 0:
                        # first 512-token chunk of x across all contraction
                        # chunks, so the first matmul chain can start early
                        for ck in range(NCK):
                            nc.sync.dma_start(xt_sb[:, ck, 0:512],
                                              xt[ck, :, 0:512])
                        nc.sync.dma_start(tri_sb[:], tri[:])
                for h4 in range(1, NI):
                    ts = slice(512 * h4, 512 * h4 + 512)
                    for ck in range(NCK):
                        nc.sync.dma_start(xt_sb[:, ck, ts], xt[ck, :, ts])
                for hp in range(HP):
                    nc.sync.dma_start(wp_sb[:, hp, :], wp[hp])

                for hp in range(HP):
                    wq_h, wk_h, wv_h = w_tiles[hp]
                    for i in range(NI):
                        ts = slice(512 * i, 512 * i + 512)
                        pq = ps_a.tile([128, 512], F32, tag="mm")
                        for ck in range(NCK):
                            nc.tensor.matmul(pq[:], wq_h[:, ck, :],
                                             xt_sb[:, ck, ts],
                                             start=(ck == 0),
                                             stop=(ck == NCK - 1))
                        nc.scalar.activation(q_sb[:, hp, ts], pq[:], COPY)
                        pk = ps_a.tile([128, 512], F32, tag="mm")
                        for ck in range(NCK):
                            nc.tensor.matmul(pk[:], wk_h[:, ck, :],
                                             xt_sb[:, ck, ts],
                                             start=(ck == 0),
                                             stop=(ck == NCK - 1))
                        nc.scalar.activation(k_sb[:, hp, ts], pk[:], COPY)
                        pv = ps_a.tile([128, 512], F32, tag="mm")
                        for ck in range(NCK):
                            nc.tensor.matmul(pv[:], wv_h[:, ck, :],
                                             xt_sb[:, ck, ts],
                                             start=(ck == 0),
                                             stop=(ck == NCK - 1))
                        vt_bf = vtp.tile([128, 512], BF16, tag="vt")
                        nc.scalar.activation(vt_bf[:], pv[:], COPY)
                        for f in range(4):
                            pt = ps_tr.tile([128, 128], BF16, tag="tr")
                            nc.tensor.transpose(pt[:],
                                                vt_bf[:, 128 * f:128 * f + 128],
                                                id_sb[:])
                            nc.vector.tensor_copy(
                                v_sb[:, hp, 4 * i + f, :, 0:64],
                                pt[:].rearrange("p (h d) -> p h d", h=2))

            # ---- phases B + C ----
            with (
                tc.tile_pool(name="att", bufs=12) as attp,
                tc.tile_pool(name="nrm", bufs=2) as nrm,
                tc.tile_pool(name="otp", bufs=2) as otp,
                tc.tile_pool(name="ps_s", bufs=2, space="PSUM") as ps_s,
                tc.tile_pool(name="ps_acc", bufs=2, space="PSUM") as ps_acc,
            ):
                def proj(t):
                    ysl = slice(128 * t, 128 * t + 128)
                    po = ps_acc.tile([128, 1024], F32, tag="acc", name="po")
                    for ch in range(2):
                        cs = slice(512 * ch, 512 * ch + 512)
                        for hp in range(HP):
                            nc.tensor.matmul(po[:, cs], y_sb[:, hp, ysl],
                                             wp_sb[:, hp, cs],
                                             start=(hp == 0),
                                             stop=(hp == HP - 1))
                    ot = otp.tile([128, C], BF16, tag="ot", name="ot")
                    nc.vector.tensor_copy(ot[:], po[:])
                    nc.sync.dma_start(out[ysl, :], ot[:])

                for j in range(NI):
                    tqs = slice(512 * j, 512 * j + 512)
                    ntk = 4 * j + 4
                    for hp in range(HP):
                        pyd = ps_acc.tile([128, 1024], F32, tag="acc",
                                          name="pyd")
                        for tkb in range(ntk):
                            ks = slice(128 * tkb, 128 * tkb + 128)
                            pss = ps_s.tile([128, 1024], F32, tag="s")
                            nc.tensor.matmul(pss[:, 0:512], k_sb[0:64, hp, ks],
                                             q_sb[0:64, hp, tqs],
                                             start=True, stop=True,
                                             tile_position=(0, 0))
                            nc.tensor.matmul(pss[:, 512:1024],
                                             k_sb[64:128, hp, ks],
                                             q_sb[64:128, hp, tqs],
                                             start=True, stop=True,
                                             tile_position=(64, 0))
                            att = attp.tile([128, 2, 512], BF16, tag="att")
                            r = tkb - 4 * j
                            if r < 0:
                                nc.scalar.activation(
                                    att[:],
                                    pss[:].rearrange("p (h t) -> p h t", h=2),
                                    EXP, scale=0.125)
                            else:
                                if r > 0:
                                    nc.vector.memset(att[:, :, 0:128 * r], 0.0)
                                nc.scalar.activation(
                                    att[:, :, 128 * r:512],
                                    pss[:].rearrange("p (h t) -> p h t",
                                                     h=2)[:, :, 128 * r:512],
                                    EXP, scale=0.125)
                                nc.vector.tensor_mul(
                                    att[:, :, 128 * r:128 * r + 128],
                                    att[:, :, 128 * r:128 * r + 128],
                                    tri_sb[:].rearrange("p (h t) -> p h t",
                                                        h=2))
                            st = (tkb == 0)
                            sp = (tkb == ntk - 1)
                            nc.tensor.matmul(pyd[0:65, 0:512],
                                             v_sb[:, hp, tkb, 0, :],
                                             att[:, 0, :], start=st, stop=sp)
                            nc.tensor.matmul(pyd[0:65, 512:1024],
                                             v_sb[:, hp, tkb, 1, :],
                                             att[:, 1, :], start=st, stop=sp)
                        # drain PSUM fast, then normalize off the PE path
                        yu = nrm.tile([65, 1024], F32, tag="yu", name="yu")
                        nc.scalar.activation(yu[:], pyd[0:65, :], COPY)
                        den0 = nrm.tile([1, 1024], F32, tag="den0",
                                        name="den0")
                        nc.gpsimd.dma_start(den0[:], yu[64:65, :])
                        rec0 = nrm.tile([1, 1024], F32, tag="rec0",
                                        name="rec0")
                        nc.vector.reciprocal_approx_fast(rec0[:], den0[:])
                        dT = nrm.tile([64, 1024], F32, tag="dT", name="dT")
                        nc.gpsimd.partition_broadcast(dT[:], rec0[0:1, :])
                        nc.vector.tensor_mul(y_sb[0:64, hp, tqs],
                                             yu[0:64, 0:512], dT[:, 0:512])
                        yb = nrm.tile([64, 512], BF16, tag="yb", name="yb")
                        nc.vector.tensor_mul(yb[:], yu[0:64, 512:1024],
                                             dT[:, 512:1024])
                        nc.gpsimd.dma_start(y_sb[64:128, hp, tqs], yb[:])
                        # interleave the previous block's projection so the
                        # PE never waits on the normalize chain
                        if j > 0 and hp < 2:
                            proj(4 * (j - 1) + 2 * hp)
                            proj(4 * (j - 1) + 2 * hp + 1)
                for t in range(4 * (NI - 1), 4 * NI):
                    proj(t)

    nc.compile()
    return nc


def make_inputs(x_b, w_qkv, w_proj, g, HL=8):
    """Host-side prep of one core's input map.

    x_b: [T, C] fp32 (one batch), g: head-group index (0 or 1).
    """
    import ml_dtypes
    T, C = x_b.shape
    D = 64
    NCK = C // 128
    HP = HL // 2
    h0 = g * HL * D
    bf = ml_dtypes.bfloat16
    xt = np.ascontiguousarray(x_b.T.reshape(NCK, 128, T)).astype(bf)
    wqkv = np.empty((3, 128, HP, NCK, 128), dtype=np.float32)
    for kind in range(3):
        blk = w_qkv[:, kind * C + h0:kind * C + h0 + HL * D]
        wqkv[kind] = blk.reshape(NCK, 128, HP, 128).transpose(1, 2, 0, 3)
    wqkv = np.ascontiguousarray(wqkv).astype(bf)
    wpz = np.ascontiguousarray(
        w_proj[h0:h0 + HL * D, :].reshape(HP, 128, C)).astype(bf)
    t1 = np.triu(np.ones((128, 128), dtype=np.float32))
    tri = np.concatenate([t1, t1], axis=1).astype(bf)
    ident = np.eye(128, dtype=np.float32).astype(bf)
    return {"xt": xt, "wqkv": wqkv, "wp": wpz, "tri": tri, "ident": ident}


_NC_CACHE = {}


def kernel(x, w_qkv, w_proj):
    import numpy as np
    from concourse.bass_utils import run_bass_kernel_spmd

    x = np.ascontiguousarray(np.asarray(x, dtype=np.float32))
    w_qkv = np.ascontiguousarray(np.asarray(w_qkv, dtype=np.float32))
    w_proj = np.ascontiguousarray(np.asarray(w_proj, dtype=np.float32))
    B, T, C = x.shape

    key = (T, C)
    if key not in _NC_CACHE:
        _NC_CACHE[key] = build(T=T, HL=8, C=C)
    nc = _NC_CACHE[key]

    in_maps = [make_inputs(x[c // 2], w_qkv, w_proj, c % 2, HL=8)
               for c in range(8)]
    res = run_bass_kernel_spmd(nc, in_maps, core_ids=list(range(8)),
                               trace=False)

    out = np.zeros((B, T, C), dtype=np.float32)
    for c in range(8):
        out[c // 2] += np.asarray(res.results[c]["out"], dtype=np.float32)
    return out


# revision 8
# speedup vs baseline: 1.2351x; 1.0270x over previous
"""Causal self-attention kernel for 8 Trainium2 NeuronCores (Bass/Tile).

Problem: y = CausalSelfAttention(x) with B=4, T=2048, C=1024, 16 heads.
Sharding: 8 cores = 4 batches x 2 head-groups (8 heads each); each core
computes its batch's attention for its head group plus the partial output
projection; the host sums the two partials per batch.

v2: all-bf16 inputs, Act-engine PSUM drains, fast reciprocal normalize,
proj interleaved into the attention j-loop, PSUM ring shared yd/proj.
"""


import numpy as np
import concourse.bass as bass
import concourse.tile as tile
from concourse import mybir, bacc

F32 = mybir.dt.float32
BF16 = mybir.dt.bfloat16
EXP = mybir.ActivationFunctionType.Exp
COPY = mybir.ActivationFunctionType.Copy


def build(T=2048, HL=8, C=1024):
    """Build the per-core SPMD program. HL = heads handled by this core."""
    D = 64
    HP = HL // 2               # head pairs
    NCK = C // 128             # contraction chunks for qkv
    NI = T // 512              # 512-wide token blocks
    NTK = T // 128             # 128-wide key blocks

    nc = bacc.Bacc("TRN2", debug=False, num_devices=8)

    xt = nc.dram_tensor("xt", [NCK, 128, T], BF16, kind="ExternalInput")
    wqkv = nc.dram_tensor("wqkv", [3, 128, HP, NCK, 128], BF16,
                          kind="ExternalInput")
    wp = nc.dram_tensor("wp", [HP, 128, C], BF16, kind="ExternalInput")
    tri = nc.dram_tensor("tri", [128, 256], BF16, kind="ExternalInput")
    ident = nc.dram_tensor("ident", [128, 128], BF16, kind="ExternalInput")
    out = nc.dram_tensor("out", [T, C], BF16, kind="ExternalOutput")

    with tile.TileContext(nc) as tc:
        with tc.tile_pool(name="persist", bufs=1) as pers:
            id_sb = pers.tile([128, 128], BF16, tag="ident")
            nc.sync.dma_start(id_sb[:], ident[:])
            tri_sb = pers.tile([128, 256], BF16, tag="tri")
            wp_sb = pers.tile([128, HP, C], BF16, tag="wp")
            q_sb = pers.tile([128, HP, T], BF16, tag="q")
            k_sb = pers.tile([128, HP, T], BF16, tag="k")
            # v^T per key block: [keys, hp, tkb, head, 64 dims + ones col]
            v_sb = pers.tile([128, HP, NTK, 2, 65], BF16, tag="v")
            y_sb = pers.tile([128, HP, T], BF16, tag="y")
            nc.vector.memset(v_sb[:, :, :, :, 64:65], 1.0)

            # ---- phase A: q^T, k^T, v for all head pairs ----
            with (
                tc.tile_pool(name="xtp", bufs=1) as xtp,
                tc.tile_pool(name="wst", bufs=3) as wst,
                tc.tile_pool(name="vtb", bufs=2) as vtp,
                tc.tile_pool(name="ps_a", bufs=6, space="PSUM") as ps_a,
                tc.tile_pool(name="ps_tr", bufs=2, space="PSUM") as ps_tr,
            ):
                xt_sb = xtp.tile([128, NCK, T], BF16, tag="xt")
                w_tiles = []
                for hp in range(HP):
                    wq_h = wst.tile([128, NCK, 128], BF16, tag=f"w{hp}",
                                    name=f"wq{hp}")
                    wk_h = wst.tile([128, NCK, 128], BF16, tag=f"w{hp}",
                                    name=f"wk{hp}")
                    wv_h = wst.tile([128, NCK, 128], BF16, tag=f"w{hp}",
                                    name=f"wv{hp}")
                    nc.sync.dma_start(wq_h[:], wqkv[0, :, hp])
                    nc.sync.dma_start(wk_h[:], wqkv[1, :, hp])
                    nc.sync.dma_start(wv_h[:], wqkv[2, :, hp])
                    w_tiles.append((wq_h, wk_h, wv_h))
                    if hp == 0:
                        # all of x next: head pair 0 consumes every token
                        # chunk before the other head pairs' weights matter
                        for h4 in range(NI):
                            ts = slice(512 * h4, 512 * h4 + 512)
                            for ck in range(NCK):
                                nc.sync.dma_start(xt_sb[:, ck, ts],
                                                  xt[ck, :, ts])
                        nc.sync.dma_start(tri_sb[:], tri[:])
                for hp in range(HP):
                    nc.sync.dma_start(wp_sb[:, hp, :], wp[hp])

                for hp in range(HP):
                    wq_h, wk_h, wv_h = w_tiles[hp]
                    for i in range(NI):
                        ts = slice(512 * i, 512 * i + 512)
                        pq = ps_a.tile([128, 512], F32, tag="mm")
                        for ck in range(NCK):
                            nc.tensor.matmul(pq[:], wq_h[:, ck, :],
                                             xt_sb[:, ck, ts],
                                             start=(ck == 0),
                                             stop=(ck == NCK - 1))
                        nc.scalar.activation(q_sb[:, hp, ts], pq[:], COPY)
                        pk = ps_a.tile([128, 512], F32, tag="mm")
                        for ck in range(NCK):
                            nc.tensor.matmul(pk[:], wk_h[:, ck, :],
                                             xt_sb[:, ck, ts],
                                             start=(ck == 0),
                                             stop=(ck == NCK - 1))
                        nc.scalar.activation(k_sb[:, hp, ts], pk[:], COPY)
                        pv = ps_a.tile([128, 512], F32, tag="mm")
                        for ck in range(NCK):
                            nc.tensor.matmul(pv[:], wv_h[:, ck, :],
                                             xt_sb[:, ck, ts],
                                             start=(ck == 0),
                                             stop=(ck == NCK - 1))
                        vt_bf = vtp.tile([128, 512], BF16, tag="vt")
                        nc.scalar.activation(vt_bf[:], pv[:], COPY)
                        for f in range(4):
                            pt = ps_tr.tile([128, 128], BF16, tag="tr")
                            nc.tensor.transpose(pt[:],
                                                vt_bf[:, 128 * f:128 * f + 128],
                                                id_sb[:])
                            nc.vector.tensor_copy(
                                v_sb[:, hp, 4 * i + f, :, 0:64],
                                pt[:].rearrange("p (h d) -> p h d", h=2))

            # ---- phases B + C ----
            with (
                tc.tile_pool(name="att", bufs=12) as attp,
                tc.tile_pool(name="nrm", bufs=2) as nrm,
                tc.tile_pool(name="otp", bufs=2) as otp,
                tc.tile_pool(name="ps_s", bufs=2, space="PSUM") as ps_s,
                tc.tile_pool(name="ps_acc", bufs=2, space="PSUM") as ps_acc,
            ):
                def proj(t):
                    ysl = slice(128 * t, 128 * t + 128)
                    po = ps_acc.tile([128, 1024], F32, tag="acc", name="po")
                    for ch in range(2):
                        cs = slice(512 * ch, 512 * ch + 512)
                        for hp in range(HP):
                            nc.tensor.matmul(po[:, cs], y_sb[:, hp, ysl],
                                             wp_sb[:, hp, cs],
                                             start=(hp == 0),
                                             stop=(hp == HP - 1))
                    ot = otp.tile([128, C], BF16, tag="ot", name="ot")
                    nc.vector.tensor_copy(ot[:], po[:])
                    nc.sync.dma_start(out[ysl, :], ot[:])

                def scores(hp, j, tkb):
                    """Emit score matmuls + exp (+ causal mask) for one
                    128-key block; returns the softmaxed att tile."""
                    tqs = slice(512 * j, 512 * j + 512)
                    ks = slice(128 * tkb, 128 * tkb + 128)
                    pss = ps_s.tile([128, 1024], F32, tag="s", name="pss")
                    nc.tensor.matmul(pss[:, 0:512], k_sb[0:64, hp, ks],
                                     q_sb[0:64, hp, tqs],
                                     start=True, stop=True,
                                     tile_position=(0, 0))
                    nc.tensor.matmul(pss[:, 512:1024],
                                     k_sb[64:128, hp, ks],
                                     q_sb[64:128, hp, tqs],
                                     start=True, stop=True,
                                     tile_position=(64, 0))
                    att = attp.tile([128, 2, 512], BF16, tag="att")
                    r = tkb - 4 * j
                    if r < 0:
                        nc.scalar.activation(
                            att[:],
                            pss[:].rearrange("p (h t) -> p h t", h=2),
                            EXP, scale=0.125)
                    else:
                        if r > 0:
                            nc.vector.memset(att[:, :, 0:128 * r], 0.0)
                        nc.scalar.activation(
                            att[:, :, 128 * r:512],
                            pss[:].rearrange("p (h t) -> p h t",
                                             h=2)[:, :, 128 * r:512],
                            EXP, scale=0.125)
                        nc.vector.tensor_mul(
                            att[:, :, 128 * r:128 * r + 128],
                            att[:, :, 128 * r:128 * r + 128],
                            tri_sb[:].rearrange("p (h t) -> p h t", h=2))
                    return att

                for j in range(NI):
                    tqs = slice(512 * j, 512 * j + 512)
                    ntk = 4 * j + 4
                    for hp in range(HP):
                        pyd = ps_acc.tile([128, 1024], F32, tag="acc",
                                          name="pyd")
                        # software pipeline: scores(n+1) issue ahead of AV(n)
                        # so the PE never idles waiting for exp(n)
                        att_cur = scores(hp, j, 0)
                        for tkb in range(ntk):
                            att_nxt = (scores(hp, j, tkb + 1)
                                       if tkb + 1 < ntk else None)
                            st = (tkb == 0)
                            sp = (tkb == ntk - 1)
                            nc.tensor.matmul(pyd[0:65, 0:512],
                                             v_sb[:, hp, tkb, 0, :],
                                             att_cur[:, 0, :],
                                             start=st, stop=sp)
                            nc.tensor.matmul(pyd[0:65, 512:1024],
                                             v_sb[:, hp, tkb, 1, :],
                                             att_cur[:, 1, :],
                                             start=st, stop=sp)
                            att_cur = att_nxt
                        # drain PSUM fast, then normalize off the PE path
                        yu = nrm.tile([65, 1024], F32, tag="yu", name="yu")
                        nc.vector.tensor_copy(yu[:], pyd[0:65, :])
                        den0 = nrm.tile([1, 1024], F32, tag="den0",
                                        name="den0")
                        nc.gpsimd.dma_start(den0[:], yu[64:65, :])
                        rec0 = nrm.tile([1, 1024], F32, tag="rec0",
                                        name="rec0")
                        nc.vector.reciprocal_approx_fast(rec0[:], den0[:])
                        dT = nrm.tile([64, 1024], F32, tag="dT", name="dT")
                        nc.gpsimd.partition_broadcast(dT[:], rec0[0:1, :])
                        nc.vector.tensor_mul(y_sb[0:64, hp, tqs],
                                             yu[0:64, 0:512], dT[:, 0:512])
                        yb = nrm.tile([64, 512], BF16, tag="yb", name="yb")
                        nc.vector.tensor_mul(yb[:], yu[0:64, 512:1024],
                                             dT[:, 512:1024])
                        nc.gpsimd.dma_start(y_sb[64:128, hp, tqs], yb[:])
                        # interleave the previous block's projection so the
                        # PE never waits on the normalize chain
                        if j > 0 and hp < 2:
                            proj(4 * (j - 1) + 2 * hp)
                            proj(4 * (j - 1) + 2 * hp + 1)
                for t in range(4 * (NI - 1), 4 * NI):
                    proj(t)

    nc.compile()
    return nc


def make_inputs(x_b, w_qkv, w_proj, g, HL=8):
    """Host-side prep of one core's input map.

    x_b: [T, C] fp32 (one batch), g: head-group index (0 or 1).
    """
    import ml_dtypes
    T, C = x_b.shape
    D = 64
    NCK = C // 128
    HP = HL // 2
    h0 = g * HL * D
    bf = ml_dtypes.bfloat16
    xt = np.ascontiguousarray(x_b.T.reshape(NCK, 128, T)).astype(bf)
    wqkv = np.empty((3, 128, HP, NCK, 128), dtype=np.float32)
    for kind in range(3):
        blk = w_qkv[:, kind * C + h0:kind * C + h0 + HL * D]
        wqkv[kind] = blk.reshape(NCK, 128, HP, 128).transpose(1, 2, 0, 3)
    wqkv = np.ascontiguousarray(wqkv).astype(bf)
    wpz = np.ascontiguousarray(
        w_proj[h0:h0 + HL * D, :].reshape(HP, 128, C)).astype(bf)
    t1 = np.triu(np.ones((128, 128), dtype=np.float32))
    tri = np.concatenate([t1, t1], axis=1).astype(bf)
    ident = np.eye(128, dtype=np.float32).astype(bf)
    return {"xt": xt, "wqkv": wqkv, "wp": wpz, "tri": tri, "ident": ident}


_NC_CACHE = {}


def kernel(x, w_qkv, w_proj):
    import numpy as np
    from concourse.bass_utils import run_bass_kernel_spmd

    x = np.ascontiguousarray(np.asarray(x, dtype=np.float32))
    w_qkv = np.ascontiguousarray(np.asarray(w_qkv, dtype=np.float32))
    w_proj = np.ascontiguousarray(np.asarray(w_proj, dtype=np.float32))
    B, T, C = x.shape

    key = (T, C)
    if key not in _NC_CACHE:
        _NC_CACHE[key] = build(T=T, HL=8, C=C)
    nc = _NC_CACHE[key]

    in_maps = [make_inputs(x[c // 2], w_qkv, w_proj, c % 2, HL=8)
               for c in range(8)]
    res = run_bass_kernel_spmd(nc, in_maps, core_ids=list(range(8)),
                               trace=False)

    out = np.zeros((B, T, C), dtype=np.float32)
    for c in range(8):
        out[c // 2] += np.asarray(res.results[c]["out"], dtype=np.float32)
    return out
